# revision 11
# baseline (speedup 1.0000x reference)
"""Bass/Trainium2 kernel for nn_LocalLoss (segment-mean prototype softmax loss).

reference math:
    sums = segment_sum(x, idx, G); v = l2_normalize(sums)   # counts cancel
    xn = l2_normalize(x); logits = xn @ v.T / beta
    loss = mean_n( logsumexp_g(logits[n,:]) - logits[n, idx[n]] )

Strategy (8 cores, data-parallel over N):
  phase 1: per 128-row chunk, one-hot segsum matmul (bf16) accumulating
           segsum(x)^T in PSUM, fused with an identity matmul producing x^T
           tiles; ACT computes row norms (square+accumulate).
  allreduce: 1 MB partial sums across 8 cores.
  v-finalize: column norms via ones-matmul, v^T (bf16) for phase 2 and a
           natural-layout v table in DRAM for row gathers.
  phase 2: logits = x^T . v^T in PSUM; ACT exp(scale=s_n/beta) with fused
           row-sum accumulation (|logits|<=10 so no max subtraction);
           picked term via indirect-DMA gather of v[idx] + fused
           multiply-reduce on DVE. Host sums the 8 per-core partials.
"""

import numpy as np

import concourse.bass as bass
import concourse.tile as tile
import concourse.mybir as mybir
from concourse.bass import IndirectOffsetOnAxis
from concourse.bass_utils import run_bass_kernel_spmd
from concourse.masks import make_identity

N_CORES = 8
N, D, G = 65536, 256, 1024
NS = N // N_CORES          # 8192 rows per core
C = NS // 128              # 64 chunks of 128 rows
BETA = 0.1

F32 = mybir.dt.float32
BF16 = mybir.dt.bfloat16
I32 = mybir.dt.int32
AF = mybir.ActivationFunctionType
ALU = mybir.AluOpType

_PATCHED = False


def _patch_tile_drain():
    """walrus on this image only supports ONE sync-wait per CTRL instruction;
    Tile's tail drain carries many. Split them across single-wait drains."""
    global _PATCHED
    if _PATCHED:
        return
    _PATCHED = True

    def _split_drain_and_barrier(self, tick_clock, wait_clock):
        nc = self.nc
        drain_inst = nc.sync.drain()
        wait_clock.add_sem_waits(
            drain_inst.ins, tile.ScopedClock({None: tick_clock.global_clock})
        )
        si = drain_inst.ins.sync_info
        waits = list(si.on_wait or []) if si is not None else []
        if len(waits) > 1:
            si.on_wait = [waits[0]]
            for w in waits[1:]:
                extra = nc.sync.drain()
                extra.ins.sync_info = mybir.SyncInfo(on_wait=[w], on_update=[])
        nc.all_engine_barrier()
        popped = nc._tile_sem_poison_stack.pop()
        assert popped is self._sem_poison
        nc.clear_and_free_semaphores(list(self.sems.allocated().values()))
        nc.all_engine_barrier()

    tile.TileContext._drain_and_barrier = _split_drain_and_barrier


def _split_waits(nc):
    """walrus codegen here accepts only ONE sync-wait per instruction; move
    extra waits onto preceding same-engine NoOps."""
    k = 0
    for fn in nc.m.functions:
        for bb in fn.blocks:
            new_insts = []
            for inst in bb.instructions:
                si = inst.sync_info
                waits = list(si.on_wait) if (si is not None and si.on_wait) else []
                if len(waits) > 1:
                    for w in waits[:-1]:
                        nop = mybir.InstNoOp(name=f"wsplit-{k}", ins=[], outs=[])
                        k += 1
                        nop.engine = inst.engine
                        nop.sync_info = mybir.SyncInfo(on_wait=[w], on_update=[])
                        new_insts.append(nop)
                    si.on_wait = [waits[-1]]
                new_insts.append(inst)
            bb.instructions[:] = new_insts


def build_kernel() -> bass.Bass:
    nc = bass.Bass(num_devices=N_CORES)

    x_in = nc.dram_tensor("x", [NS, D], F32, kind="ExternalInput")
    idx_in = nc.dram_tensor("idxT", [128, C], I32, kind="ExternalInput")
    loss_out = nc.dram_tensor("loss", [1, 1], F32, kind="ExternalOutput")

    vn_dram = nc.dram_tensor("vn_nat", [G, D], BF16)        # gather table
    inv_dram = nc.dram_tensor("inv_row", [1, G], F32)       # bcast bounce

    # x rows n = c*128 + p  ->  [p, c, d]
    x_src = x_in[:].rearrange("(c p) d -> p c d", p=128)

    with tile.TileContext(nc) as tc:
        with (
            tc.tile_pool(name="persist", bufs=1) as pp,
            tc.tile_pool(name="work", bufs=3) as wp,
            tc.tile_pool(name="dram", bufs=1, space="DRAM") as dp,
        ):
            # ---------- persistent tiles ----------
            x_sb = pp.tile([128, C * D], BF16, tag="x_sb")          # 4 MB
            xt_sb = pp.tile([128, 2 * C * 128], BF16, tag="xt_sb")  # 4 MB
            idx_sb = pp.tile([128, C], I32, tag="idx_sb")
            idx_f32 = pp.tile([128, C], F32, tag="idx_f32")
            iota_g = pp.tile([128, G], F32, tag="iota_g")
            ident_bf = pp.tile([128, 128], BF16, tag="ident_bf")
            ident_f32 = pp.tile([128, 128], F32, tag="ident_f32")
            ones_f32 = pp.tile([128, 1], F32, tag="ones_f32")
            invN = pp.tile([128, 1], F32, tag="invN")
            sumsq = pp.tile([128, C], F32, tag="sumsq")
            s_beta = pp.tile([128, C], F32, tag="s_beta")
            sumexp = pp.tile([128, C], F32, tag="sumexp")
            tdot = pp.tile([128, C], F32, tag="tdot")
            sums_sb = pp.tile([128, 2 * G], F32, tag="sums_sb")     # 1 MB
            sums_red = pp.tile([128, 2 * G], F32, tag="sums_red")   # 1 MB
            vnT_sb = pp.tile([128, 2 * G], BF16, tag="vnT_sb")      # 512 KB
            bc_sb = pp.tile([128, G], F32, tag="bc_sb")
            vn_nat = pp.tile([128, 8 * D], BF16, tag="vn_nat")      # 512 KB

            cc_in = dp.tile([128, 2 * G], F32, tag="cc_in")
            cc_out = dp.tile([128, 2 * G], F32, tag="cc_out")

            # ---------- constants / input loads ----------
            make_identity(nc, ident_bf[:])
            make_identity(nc, ident_f32[:])
            nc.vector.memset(ones_f32[:], 1.0)
            nc.vector.memset(invN[:], 1.0 / N)
            nc.gpsimd.iota(iota_g[:], pattern=[[1, G]], base=0,
                           channel_multiplier=0,
                           allow_small_or_imprecise_dtypes=True)
            nc.sync.dma_start(out=idx_sb[:], in_=idx_in[:])
            nc.vector.tensor_copy(out=idx_f32[:], in_=idx_sb[:])
            for b in range(8):  # 8 x 1MB cast-DMAs (f32 -> bf16)
                nc.gpsimd.dma_start(
                    out=x_sb[:].rearrange("p (c d) -> p c d", d=D)
                    [:, b * 8:(b + 1) * 8, :],
                    in_=x_src[:, b * 8:(b + 1) * 8, :],
                )

            x_cmk = x_sb[:].rearrange("p (c m k) -> p c m k", c=C, m=2)
            x_cd = x_sb[:].rearrange("p (c d) -> p c d", d=D)
            xt_mck = xt_sb[:].rearrange("p (m c k) -> p m c k", m=2, c=C)

            ph1 = tc.alloc_tile_pool(name="psum_p1", bufs=1, space="PSUM")
            ph1r = tc.alloc_tile_pool(name="psum_p1r", bufs=2, space="PSUM")
            psum_sums = [
                ph1.tile([128, G], F32, tag=f"sums{m}", name=f"psum_sums{m}")
                for m in (0, 1)
            ]

            # ================= phase 1 =================
            for c in range(C):
                onehot = wp.tile([128, G], BF16, tag="onehot")
                nc.vector.tensor_scalar(
                    out=onehot[:], in0=iota_g[:], scalar1=idx_f32[:, c:c + 1],
                    scalar2=None, op0=ALU.is_equal,
                )
                sq_scr = wp.tile([128, D], BF16, tag="sq_scr")
                nc.scalar.activation(
                    out=sq_scr[:], in_=x_cd[:, c, :], func=AF.Square,
                    accum_out=sumsq[:, c:c + 1],
                )
                psum_xt = ph1r.tile([128, 2 * 128], F32, tag="psum_xt")
                for m in (0, 1):
                    lhsT = x_cmk[:, c, m, :]
                    for gb in (0, 1):
                        nc.tensor.matmul(
                            out=psum_sums[m][:, gb * 512:(gb + 1) * 512],
                            lhsT=lhsT, rhs=onehot[:, gb * 512:(gb + 1) * 512],
                            start=(c == 0), stop=(c == C - 1),
                            skip_group_check=True,
                        )
                    nc.tensor.matmul(
                        out=psum_xt[:, m * 128:(m + 1) * 128],
                        lhsT=lhsT, rhs=ident_bf[:],
                        start=True, stop=True, skip_group_check=True,
                    )
                nc.scalar.copy(
                    out=xt_mck[:, :, c, :],
                    in_=psum_xt[:].rearrange("p (m k) -> p m k", m=2),
                )

            # s_beta = 1/(beta*||x_n||)
            nrm_x = wp.tile([128, C], F32, tag="nrm_x")
            nc.scalar.activation(out=nrm_x[:], in_=sumsq[:], func=AF.Sqrt)
            nc.vector.tensor_scalar_max(out=nrm_x[:], in0=nrm_x[:], scalar1=1e-12)
            s_x = wp.tile([128, C], F32, tag="s_x")
            nc.vector.reciprocal(out=s_x[:], in_=nrm_x[:])
            nc.vector.tensor_scalar_mul(out=s_beta[:], in0=s_x[:], scalar1=1.0 / BETA)

            # ---------- allreduce of segment sums ----------
            for m in (0, 1):
                nc.scalar.copy(out=sums_sb[:, m * G:(m + 1) * G], in_=psum_sums[m][:])
            nc.gpsimd.dma_start(out=cc_in[:], in_=sums_sb[:])
            nc.gpsimd.collective_compute(
                "AllReduce", ALU.add,
                replica_groups=[list(range(N_CORES))],
                ins=[cc_in.opt()], outs=[cc_out.opt()],
            )
            nc.sync.dma_start(out=sums_red[:], in_=cc_out[:])
            ph1r.release()
            ph1.release()

            # ---------- v-finalize ----------
            fin = tc.alloc_tile_pool(name="psum_fin", bufs=2, space="PSUM")
            sqs = wp.tile([128, 2 * G], F32, tag="sqs")
            nc.scalar.activation(out=sqs[:], in_=sums_red[:], func=AF.Square)
            psum_nsq = fin.tile([128, 8], F32, tag="psum_nsq", bufs=1)
            for b in range(8):
                for m in (0, 1):
                    nc.tensor.matmul(
                        out=psum_nsq[:, b:b + 1],
                        lhsT=sqs[:, m * G + b * 128: m * G + (b + 1) * 128],
                        rhs=ones_f32[:],
                        start=(m == 0), stop=(m == 1), skip_group_check=True,
                    )
            nrm_v = wp.tile([128, 8], F32, tag="nrm_v")
            nc.scalar.activation(out=nrm_v[:], in_=psum_nsq[:], func=AF.Sqrt)
            nc.vector.tensor_scalar_max(out=nrm_v[:], in0=nrm_v[:], scalar1=1e-12)
            inv_v = wp.tile([128, 8], F32, tag="inv_v")
            nc.vector.reciprocal(out=inv_v[:], in_=nrm_v[:])

            # broadcast 1/||v_g|| to all partitions via DRAM roundtrip
            psum_it = fin.tile([8, 128], F32, tag="psum_it", bufs=1)
            nc.tensor.transpose(out=psum_it[:], in_=inv_v[:], identity=ident_f32[:])
            invT_sb = wp.tile([8, 128], F32, tag="invT_sb")
            nc.scalar.copy(out=invT_sb[:], in_=psum_it[:])
            nc.sync.dma_start(
                out=inv_dram[0:1, :].rearrange("o (a k) -> (o a) k", a=8),
                in_=invT_sb[:],
            )
            nc.sync.dma_start(out=bc_sb[:], in_=inv_dram[0:1, :].to_broadcast([128, G]))
            for m in (0, 1):
                nc.vector.tensor_tensor(
                    out=vnT_sb[:, m * G:(m + 1) * G],
                    in0=sums_red[:, m * G:(m + 1) * G], in1=bc_sb[:],
                    op=ALU.mult,
                )

            # natural-layout vn table for row gathers
            for b in range(8):
                for m in (0, 1):
                    psum_tr = fin.tile([128, 128], F32, tag="psum_tr")
                    nc.tensor.transpose(
                        out=psum_tr[:],
                        in_=sums_red[:, m * G + b * 128: m * G + (b + 1) * 128],
                        identity=ident_f32[:],
                    )
                    nc.vector.tensor_scalar_mul(
                        out=vn_nat[:].rearrange("p (b d) -> p b d", b=8)
                        [:, b, m * 128:(m + 1) * 128],
                        in0=psum_tr[:], scalar1=inv_v[:, b:b + 1],
                    )
            nc.sync.dma_start(
                out=vn_dram[:].rearrange("(b g) d -> g b d", g=128),
                in_=vn_nat[:].rearrange("p (b d) -> p b d", b=8),
            )

            # ================= phase 2 =================
            fin.release()
            ph2 = tc.alloc_tile_pool(name="psum_p2", bufs=2, space="PSUM")
            for c in range(C):
                psum_log = ph2.tile([128, G], F32, tag="psum_log")
                for m in (0, 1):
                    lhsT = xt_mck[:, m, c, :]
                    for gb in (0, 1):
                        nc.tensor.matmul(
                            out=psum_log[:, gb * 512:(gb + 1) * 512],
                            lhsT=lhsT,
                            rhs=vnT_sb[:, m * G + gb * 512: m * G + (gb + 1) * 512],
                            start=(m == 0), stop=(m == 1), skip_group_check=True,
                        )
                exp_scr = wp.tile([128, G], BF16, tag="exp_scr")
                nc.scalar.activation(
                    out=exp_scr[:], in_=psum_log[:], func=AF.Exp,
                    scale=s_beta[:, c:c + 1], accum_out=sumexp[:, c:c + 1],
                )
                u_t = wp.tile([128, D], BF16, tag="u_t")
                nc.gpsimd.indirect_dma_start(
                    out=u_t[:], out_offset=None, in_=vn_dram[:],
                    in_offset=IndirectOffsetOnAxis(ap=idx_sb[:, c:c + 1], axis=0),
                )
                tt_scr = wp.tile([128, D], BF16, tag="tt_scr")
                nc.vector.tensor_tensor(
                    out=tt_scr[:], in0=x_cd[:, c, :], in1=u_t[:], op=ALU.mult,
                )
                nc.vector.reduce_sum(
                    out=tdot[:, c:c + 1], in_=tt_scr[:],
                    axis=mybir.AxisListType.X,
                )

            # ---------- final reduction ----------
            logls = wp.tile([128, C], F32, tag="logls")
            nc.scalar.activation(out=logls[:], in_=sumexp[:], func=AF.Ln)
            picked = wp.tile([128, C], F32, tag="picked")
            nc.vector.tensor_tensor(out=picked[:], in0=tdot[:], in1=s_beta[:],
                                    op=ALU.mult)
            diff = wp.tile([128, C], F32, tag="diff")
            nc.vector.tensor_tensor(out=diff[:], in0=logls[:], in1=picked[:],
                                    op=ALU.subtract)
            colsum = wp.tile([128, 1], F32, tag="colsum")
            nc.vector.reduce_sum(out=colsum[:], in_=diff[:],
                                 axis=mybir.AxisListType.X)
            psum_tot = ph2.tile([1, 1], F32, tag="psum_tot", bufs=1)
            nc.tensor.matmul(out=psum_tot[:], lhsT=colsum[:], rhs=invN[:],
                             start=True, stop=True, skip_group_check=True)
            out_sb = wp.tile([1, 1], F32, tag="out_sb")
            nc.scalar.copy(out=out_sb[:], in_=psum_tot[:])
            nc.sync.dma_start(out=loss_out[:], in_=out_sb[:])
            ph2.release()

    _split_waits(nc)
    return nc


_NC_CACHE = None


def kernel(x: np.ndarray, idx: np.ndarray) -> np.ndarray:
    global _NC_CACHE
    _patch_tile_drain()
    if _NC_CACHE is None:
        _NC_CACHE = build_kernel()
    nc = _NC_CACHE

    x = np.ascontiguousarray(np.asarray(x, dtype=np.float32)).reshape(N_CORES, NS, D)
    idx_i32 = np.asarray(idx, dtype=np.int32).reshape(N_CORES, C, 128)

    in_maps = []
    for i in range(N_CORES):
        in_maps.append({
            "x": x[i],
            "idxT": np.ascontiguousarray(idx_i32[i].T),  # [128, C]
        })
    res = run_bass_kernel_spmd(nc, in_maps, core_ids=list(range(N_CORES)))
    total = np.float64(0.0)
    for r in res.results:
        total += np.float64(r["loss"][0, 0])
    return np.float32(total)


if __name__ == "__main__":
    rng = np.random.default_rng(0)
    x = rng.standard_normal((N, D)).astype(np.float32)
    idx = rng.integers(0, G, size=(N,)).astype(np.int64)
    print("loss:", kernel(x, idx))
